# revision 46
# baseline (speedup 1.0000x reference)
"""DeepGESNCell kernel for 8 TRN2 NeuronCores.

h <- tanh(wiu + L @ (h @ W_hh^T)) iterated 10x, two layers, out = [h1|h2].

Strategy (row-sharded graph parallel):
  - core c owns rows I_c = [512c, 512c+512) of L; L[I_c].T stays resident in
    SBUF for all 20 iterations -> L is read from HBM exactly once.
  - Associativity: L @ (h @ W^T) == (L @ h) @ W^T.  Per iteration each core
    computes y_c^T = (L[I_c] @ h)^T with gathered-h tiles as the stationary
    operand and resident L^T tiles as the 512-wide moving operand, 2-way
    column-tiled across the PE array (two 64-wide matmuls share a 512-cycle
    slot -> full PE width).
  - z_c = y_c @ W_hh^T + wiu_c is produced in NORMAL layout by matmuls whose
    stationary operand is an SBUF copy of the stacked py halves (a stacked
    [W_hh^T; W_hh^T] moving operand folds the even/odd k halves' sum); the
    wiu accumulation runs as two hoisted bf16 identity-matmuls (wiu split
    hi/lo) issued before the big passes so the post-matmul tail is short.

Precision: double-bf16 split (L = L_hi + L_lo, h = h_hi + h_lo, keep the
three dominant terms, f32 PSUM accumulation) -> rel err ~4.3e-4.  Single
bf16/fp16 h fails or is too marginal (simulated 9.8e-2 / 1.4e-2 vs the
2e-2 gate), so both bf16 halves of h must move every iteration.

Communication/scheduling (learned from perfetto traces of prior revisions):
  - Each collective costs ~4-5us of fixed CC-queue time (trigger chain +
    mesh rendezvous + events) on top of ~3us/512KB wire time, and the CC
    queue serializes collectives -> exactly TWO AllGathers per iteration
    (hi then lo); the lo gather and its wire time hide behind the hi
    matmul passes.  Splitting further (3-4 smaller gathers) regresses.
  - cc buffers are p-major ([128, 256] rank blocks) so stage + fetch DMAs
    move 512B contiguous lines; fetch is chunked (1,1,2,4 ranks) into
    separate tiles across both HWDGE queues so the first matmuls' weight
    loads wait only on the 64KB first chunk.
  - A PE keep-warm filler bridges the ~12us gather hole: the PE clock
    drops to 1.2GHz after an idle and takes ~3us of continuous work to
    recover, so without it each iteration's first matmul pairs run at
    roughly half speed.  The filler's stationary operand is the current
    iteration's hs_hi so the tile scheduler cannot hoist it out of the
    hole (a dependency-free filler gets pooled at kernel start, jamming
    the in-order PE queue ahead of real work).
"""

import os
import sys

import numpy as np

sys.path.insert(0, "/opt/trn_rl_repo")

import ml_dtypes

N = 4096
D_IN = 64
H = 64
MAX_ITER = 10
NCORES = 8
ROWS = N // NCORES  # 512
KT = N // 128  # 32 k-tiles over the node dim
RT = ROWS // 128  # 4 row-tiles per core chunk

_CACHE = {}
LAST_RESULTS = None

# fetch chunking: rank counts per chunk DMA (first chunk smallest so the
# first matmul's dependency clears earliest)
FETCH_CHUNKS = (1, 2, 2, 3)

# PE keep-warm filler matmuls per iteration: the PE clock drops to 1.2GHz
# after an idle and takes ~3us of continuous work to return to 2.4GHz, so
# the ~12us gather hole makes each iteration's first ~7 matmul pairs run
# at half clock.  The filler's stationary operand is the CURRENT
# iteration's hs_hi so the tile scheduler cannot hoist it out of the hole
# (a dependency-free filler gets pooled at kernel start, jamming the
# in-order PE queue ahead of real work).
WARM_N = int(os.environ.get("KERNEL_WARM_N", "50"))


def _build_nc():
    import concourse.bacc as bacc
    import concourse.mybir as mybir
    import concourse.tile as tile
    from concourse import masks

    F32 = mybir.dt.float32
    BF = mybir.dt.bfloat16
    TANH = mybir.ActivationFunctionType.Tanh

    nc = bacc.Bacc(None, target_bir_lowering=False, num_devices=NCORES)

    LTH = nc.dram_tensor("LTH", [N, ROWS], BF, kind="ExternalInput")
    LTL = nc.dram_tensor("LTL", [N, ROWS], BF, kind="ExternalInput")
    XT = nc.dram_tensor("XT", [D_IN, ROWS], F32, kind="ExternalInput")
    WIH0 = nc.dram_tensor("WIH0T", [D_IN, H], F32, kind="ExternalInput")
    WST0 = nc.dram_tensor("WHH0TS", [2 * H, H], F32, kind="ExternalInput")
    WIH1 = nc.dram_tensor("WIH1T", [H, H], F32, kind="ExternalInput")
    WST1 = nc.dram_tensor("WHH1TS", [2 * H, H], F32, kind="ExternalInput")
    OUT = nc.dram_tensor("OUT", [ROWS, 2 * H], F32, kind="ExternalOutput")

    replica = [list(range(NCORES))]

    with tile.TileContext(nc) as tc:
        with (
            tc.tile_pool(name="cpool", bufs=1) as cpool,
            tc.tile_pool(name="spool", bufs=2) as spool,
            tc.tile_pool(name="ppool", bufs=2, space="PSUM") as ppool,
            tc.tile_pool(name="dpool", bufs=2, space="DRAM") as dpool,
        ):
            ident_bf = cpool.tile([128, 128], BF)
            masks.make_identity(nc, ident_bf[:])

            out_stage = cpool.tile([128, RT, 2 * H], F32)

            # small operands first so they don't queue behind the 8MB L load
            xt = cpool.tile([D_IN, ROWS], F32)
            nc.sync.dma_start(xt[:], XT.ap())
            wih0 = cpool.tile([D_IN, H], F32)
            nc.sync.dma_start(wih0[:], WIH0.ap())
            wst0 = cpool.tile([2 * H, H], F32)
            nc.sync.dma_start(wst0[:], WST0.ap())
            wih1 = cpool.tile([H, H], F32)
            nc.sync.dma_start(wih1[:], WIH1.ap())
            wst1 = cpool.tile([2 * H, H], F32)
            nc.sync.dma_start(wst1[:], WST1.ap())

            # resident L^T shard, hi/lo: lth[p, k, m] = bf16(L[I_c[m], 128k+p])
            # chunked + spread over both HWDGE queues; k=0 chunks land first
            # so the first iteration's passes can start early.
            lth = cpool.tile([128, KT, ROWS], BF)
            ltl = cpool.tile([128, KT, ROWS], BF)
            lth_src = LTH.ap().rearrange("(k p) m -> p k m", p=128)
            ltl_src = LTL.ap().rearrange("(k p) m -> p k m", p=128)
            for q in range(4):
                ks = slice(8 * q, 8 * (q + 1))
                nc.sync.dma_start(lth[:, ks, :], lth_src[:, ks, :])
                nc.scalar.dma_start(ltl[:, ks, :], ltl_src[:, ks, :])

            def gather(hpart, tagsuf, split_stage=False):
                """AllGather one bf16 [128, RT, H] part (512KB out).

                cc layout is p-major: rank block = [128, RT*H] with 512B
                contiguous per partition.  With split_stage the input is
                staged in two DMAs so the first half rides the queue while
                the second half's casts still run."""
                cc_in = dpool.tile(
                    [128, RT, H], BF, tag="cc_in" + tagsuf, name="cc_in" + tagsuf
                )
                if split_stage:
                    nc.sync.dma_start(cc_in[:, 0:2, :], hpart[:, 0:2, :])
                    nc.sync.dma_start(cc_in[:, 2:4, :], hpart[:, 2:4, :])
                else:
                    nc.sync.dma_start(cc_in[:], hpart[:])
                cc_out = dpool.tile(
                    [NCORES, 128, RT * H],
                    BF,
                    tag="cc_out" + tagsuf,
                    name="cc_out" + tagsuf,
                    addr_space="Shared",
                )
                nc.gpsimd.collective_compute(
                    "AllGather",
                    mybir.AluOpType.bypass,
                    replica_groups=replica,
                    ins=[cc_in.opt()],
                    outs=[cc_out.opt()],
                )
                return cc_out

            def fetch(cc_out, tagsuf):
                """DRAM [NCORES, 128, RT*H] -> SBUF chunk tiles.

                Chunks of (1,1,2,4) ranks, each its OWN tile so the first
                matmuls' weight loads depend only on the 64KB first chunk;
                spread across both HWDGE queues; 512B lines."""
                src = cc_out.rearrange("r p f -> p r f")
                chunks = []
                r0 = 0
                for q, nr in enumerate(FETCH_CHUNKS):
                    hq = spool.tile(
                        [128, nr, RT, H],
                        BF,
                        tag=f"ha{tagsuf}{q}",
                        name=f"ha{tagsuf}{q}",
                    )
                    eng = nc.sync if q % 2 == 0 else nc.scalar
                    eng.dma_start(
                        hq.rearrange("p r t h -> p r (t h)"),
                        src[:, r0 : r0 + nr, :],
                    )
                    chunks.append((r0, hq))
                    r0 += nr

                def at(k):
                    # stationary [128, 64] for global k-tile k
                    r, t = k // 4, k % 4
                    for rbase, hq in reversed(chunks):
                        if r >= rbase:
                            return hq[:, r - rbase, t, :]
                    raise AssertionError

                return at

            def layer(wih, wst, xT_src, out_col):
                # wiu in normal layout (fp32): wiu_n[p, j, m] = wiu[I_c[128j+p], m]
                pw = ppool.tile([128, RT, H], F32, tag="pw", name="pw", bufs=1)
                for j in range(RT):
                    nc.tensor.matmul(
                        pw[:, j, :],
                        xT_src[:, 128 * j : 128 * (j + 1)],
                        wih[:],
                        start=True,
                        stop=True,
                    )
                # wiu split to bf16 hi/lo so the per-iteration wiu
                # accumulation matmuls are bf16 (hoistable before py without
                # leaving an fp32 accumulation group open across it) and run
                # as two F=256 matmuls instead of four fp32 F=64 ones.
                wiu_hi = spool.tile([128, RT, H], BF, tag="wiuh", name="wiu_hi")
                nc.vector.tensor_copy(wiu_hi[:], pw[:])
                wiu_lo = spool.tile([128, RT, H], BF, tag="wiul", name="wiu_lo")
                nc.vector.tensor_sub(wiu_lo[:], pw[:], wiu_hi[:])
                h_own = spool.tile([128, RT, H], F32, tag="h_own", name="h_own")
                nc.scalar.activation(h_own[:], pw[:], TANH)
                hs_hi = spool.tile([128, RT, H], BF, tag="hsh", name="hs_hi")
                nc.vector.tensor_copy(hs_hi[:], h_own[:])
                hs_lo = spool.tile([128, RT, H], BF, tag="hsl", name="hs_lo")
                nc.vector.tensor_sub(hs_lo[:], h_own[:], hs_hi[:])

                for _t in range(2, MAX_ITER + 1):
                    cc_hi = gather(hs_hi, "h")
                    cc_lo = gather(hs_lo, "l")
                    h_hi = fetch(cc_hi, "h")
                    h_lo = fetch(cc_lo, "l")
                    # pz in TWO psum tiles (row-tile pairs) so the tanh of
                    # the first half doesn't wait on the second half's
                    # matmuls (engine waits coarsen to the whole tile).
                    # Opens with the wiu accumulation (bf16, hoisted before
                    # py so the post-py tail is just yab-copy + one matmul
                    # per j).  bufs=1 is safe: the WAR on the previous
                    # iteration's readers is long satisfied by the time the
                    # PE reaches these in its queue.
                    pzs = [
                        ppool.tile([128, 2, H], F32, tag=f"pz{u}", name=f"pz{u}", bufs=1)
                        for u in range(2)
                    ]
                    for u, wiu_part in ((0, wiu_hi), (1, wiu_lo)):
                        for v in range(2):
                            nc.tensor.matmul(
                                pzs[v].rearrange("p t h -> p (t h)"),
                                ident_bf[:],
                                wiu_part[:, 2 * v : 2 * v + 2, :].rearrange(
                                    "p t h -> p (t h)"
                                ),
                                start=(u == 0), stop=False, skip_group_check=True,
                            )
                    py = ppool.tile([128, ROWS], F32, tag="py", name="py")
                    yab = spool.tile([128, ROWS], F32, tag="yab", name="yab")
                    nkk = KT // 2
                    # hi passes first (overlap the lo gather), full 512 rows
                    for pi, ltx in enumerate((lth, ltl)):
                        for kk in range(nkk):
                            k0, k1 = 2 * kk, 2 * kk + 1
                            nc.tensor.matmul(
                                py[0:64, :],
                                h_hi(k0),
                                ltx[:, k0, :],
                                start=(pi == 0 and kk == 0),
                                stop=False,
                                tile_position=(0, 0),
                                skip_group_check=True,
                            )
                            nc.tensor.matmul(
                                py[64:128, :],
                                h_hi(k1),
                                ltx[:, k1, :],
                                start=(pi == 0 and kk == 0),
                                stop=False,
                                tile_position=(0, 64),
                                skip_group_check=True,
                            )
                    # lo pass, full 512 rows
                    for kk in range(nkk):
                        k0, k1 = 2 * kk, 2 * kk + 1
                        stp = kk == nkk - 1
                        nc.tensor.matmul(
                            py[0:64, :],
                            h_lo(k0),
                            lth[:, k0, :],
                            start=False,
                            stop=stp,
                            tile_position=(0, 0),
                            skip_group_check=True,
                        )
                        nc.tensor.matmul(
                            py[64:128, :],
                            h_lo(k1),
                            lth[:, k1, :],
                            start=False,
                            stop=stp,
                            tile_position=(0, 64),
                            skip_group_check=True,
                        )
                    for j in range(RT):
                        nc.vector.tensor_copy(
                            yab[:, 128 * j : 128 * (j + 1)],
                            py[:, 128 * j : 128 * (j + 1)],
                        )
                    for j in range(RT):
                        nc.tensor.matmul(
                            pzs[j // 2][:, j % 2, :],
                            yab[:, 128 * j : 128 * (j + 1)],
                            wst[:],
                            start=False,
                            stop=True,
                            skip_group_check=True,
                        )
                    h_own = spool.tile([128, RT, H], F32, tag="h_own", name="h_own")
                    hs_hi = spool.tile([128, RT, H], BF, tag="hsh", name="hs_hi")
                    hs_lo = spool.tile([128, RT, H], BF, tag="hsl", name="hs_lo")
                    # hi casts first (they gate the next hi gather's stage);
                    # the lo subs go to the pool engine so they run off the
                    # vector queue and only gate the later lo gather.
                    for j in range(RT):
                        nc.scalar.activation(
                            h_own[:, j, :], pzs[j // 2][:, j % 2, :], TANH
                        )
                        nc.vector.tensor_copy(hs_hi[:, j, :], h_own[:, j, :])
                    for j in range(RT):
                        nc.gpsimd.tensor_sub(
                            hs_lo[:, j, :], h_own[:, j, :], hs_hi[:, j, :]
                        )
                    if _t < MAX_ITER and WARM_N > 0:
                        pwarm = ppool.tile(
                            [64, ROWS], F32, tag="warm", name="pwarm", bufs=1
                        )
                        for _ in range(WARM_N):
                            nc.tensor.matmul(
                                pwarm[:, :],
                                hs_hi[:, 0, :],
                                lth[:, 0, :],
                                start=True,
                                stop=True,
                            )

                nc.vector.tensor_copy(
                    out_stage[:, :, out_col : out_col + H], h_own[:]
                )
                return hs_hi, hs_lo

            h1_hi, h1_lo = layer(wih0, wst0, xt, 0)

            # boundary: h1^T [64, 512] fp32 for layer-1's wiu, via bf16
            # transposes of the hi/lo halves + f32 add (one PSUM operand max).
            ptr_hi = ppool.tile([64, ROWS], BF, tag="ptrh", name="ptr_hi", bufs=1)
            ptr_lo = ppool.tile([64, ROWS], BF, tag="ptrl", name="ptr_lo", bufs=1)
            for j in range(RT):
                nc.tensor.transpose(
                    ptr_hi[:, 128 * j : 128 * (j + 1)], h1_hi[:, j, :], ident_bf[:]
                )
                nc.tensor.transpose(
                    ptr_lo[:, 128 * j : 128 * (j + 1)], h1_lo[:, j, :], ident_bf[:]
                )
            h1T_hi = spool.tile([64, ROWS], F32, tag="h1Th", name="h1T_hi")
            nc.vector.tensor_copy(h1T_hi[:], ptr_hi[:])
            h1T = spool.tile([64, ROWS], F32, tag="h1T", name="h1T")
            nc.vector.tensor_add(h1T[:], h1T_hi[:], ptr_lo[:])

            layer(wih1, wst1, h1T, H)

            nc.sync.dma_start(
                OUT.ap().rearrange("(t p) h -> p t h", p=128), out_stage[:]
            )

    nc.compile()
    return nc


def _get_nc():
    if "nc" not in _CACHE:
        _CACHE["nc"] = _build_nc()
    return _CACHE["nc"]


def _ensure_ntff_hook():
    """bass_utils needs antenv.axon_hooks for trace=True under axon; the
    agent image's antenv lacks it.  Register an equivalent shim in
    sys.modules backed by ctypes calls into libaxon_pjrt.so."""
    import types

    try:
        import antenv.axon_hooks  # noqa: F401

        return
    except ImportError:
        pass
    mod = types.ModuleType("antenv.axon_hooks")
    state = {"hook": None, "tried": False}

    def set_axon_ntff_profile_hook(hook):
        state["hook"] = hook

    def get_axon_ntff_profile_hook():
        if state["hook"] is None and not state["tried"]:
            state["tried"] = True
            try:
                from trn_agent_boot.trn_boot import _ntff_profile_via_ctypes

                state["hook"] = _ntff_profile_via_ctypes(
                    "/opt/axon/libaxon_pjrt.so"
                )
            except Exception:
                state["hook"] = None
        return state["hook"]

    mod.set_axon_ntff_profile_hook = set_axon_ntff_profile_hook
    mod.get_axon_ntff_profile_hook = get_axon_ntff_profile_hook
    sys.modules["antenv.axon_hooks"] = mod


def kernel(X, L, W_ih0, W_hh0, W_ih1, W_hh1):
    global LAST_RESULTS
    _ensure_ntff_hook()
    from concourse.bass_utils import run_bass_kernel_spmd

    nc = _get_nc()
    f32 = np.float32
    bf = ml_dtypes.bfloat16

    wih0 = np.ascontiguousarray(np.asarray(W_ih0).T).astype(f32)
    wst0 = np.ascontiguousarray(
        np.concatenate([np.asarray(W_hh0).T, np.asarray(W_hh0).T], axis=0)
    ).astype(f32)
    wih1 = np.ascontiguousarray(np.asarray(W_ih1).T).astype(f32)
    wst1 = np.ascontiguousarray(
        np.concatenate([np.asarray(W_hh1).T, np.asarray(W_hh1).T], axis=0)
    ).astype(f32)

    Lf = np.asarray(L, dtype=f32)
    in_maps = []
    for c in range(NCORES):
        rows = slice(ROWS * c, ROWS * (c + 1))
        ltc = np.ascontiguousarray(Lf[rows, :].T)  # [N, ROWS] f32
        lth = ltc.astype(bf)
        ltl = (ltc - lth.astype(f32)).astype(bf)
        in_maps.append(
            {
                "LTH": lth,
                "LTL": ltl,
                "XT": np.ascontiguousarray(np.asarray(X)[rows, :].T).astype(f32),
                "WIH0T": wih0,
                "WHH0TS": wst0,
                "WIH1T": wih1,
                "WHH1TS": wst1,
            }
        )

    trace = bool(int(os.environ.get("KERNEL_TRACE", "0")))
    res = run_bass_kernel_spmd(
        nc, in_maps, core_ids=list(range(NCORES)), trace=trace
    )
    LAST_RESULTS = res
    out = np.concatenate([r["OUT"] for r in res.results], axis=0)
    return np.asarray(out, dtype=np.float32)
